# revision 7
# baseline (speedup 1.0000x reference)
"""Dilated attention (LongNet-style) Trainium2 kernel.

Problem: query/key/value (2, 8192, 12, 64) f32. Three dilation groups
(segment lengths 2048/4096/8192, dilation 1/2/4, head slices 0:4/4:8/8:12).
Each group's gather produces independent dense attention over 2048-position
dilated segments; outputs are normalized per (batch, head, channel) by the
sum over all segment positions, and divided by num_groups.

Sharding: 8 cores = 2 batches x 4 "head columns". Core c owns batch c//4 and
heads {j, 4+j, 8+j} where j = c%4 -- exactly 7 dense 2048x2048x64 attention
units per core (4 + 2 + 1 segments), perfectly balanced, with all segments of
any (batch, head) on one core so normalization needs no cross-core traffic.

Precision: the reference's x / x.sum(axis=(1,2)) normalization divides by a
nearly-cancelling sum, which amplifies independent per-element error ~300x.
bf16 matmuls (rel ~0.6) and even float32r (rel ~0.12) fail; the kernel needs
~fp32 effective precision. Scores are computed as (qh+ql)@(kh+kl)^T with
fp16 hi/lo pairs of 256*q (pre-scale keeps ql/kl out of the fp16 subnormal
range; the 2^-16 descale folds into the exp scale), giving ~22-bit scores at
2 PE cycles/row with full K=128 stacking. P stays exact fp32 out of the ACT
exp, and P@V runs as a true fp32 matmul (4 cycles/row). End-to-end rel err
~1.2e-4 vs a strict-fp32 reference.

Device kernel (same program on all 8 cores, different data):
  - inputs (per segment s, d on partitions for Q/K):
      qhh [128, 14336] fp16: rows 0-63 = qh = fp16(256*Q^T), rows 64-127 dup
      qll [128, 14336] fp16: ql = fp16(256*Q^T - qh), duplicated rows
      khl [128, 14336] fp16: rows 0-63 = kh, rows 64-127 = kl
      v1  [128, 7280] f32: V per 128-row k-block + ones column (softmax den)
  - per (chunk, k-block) unit (28 q-chunks of 512 x 16 k-blocks):
      S^T = khl_blk.T @ qhh + khl_blk.T @ qll   (PE, 1 LDW + 2 MMs, PSUM f32)
      P^T = exp(S^T * 0.125/65536)              (ACT, 3-bank spans, f32 out)
      O'[65, 512] += v1_blk.T @ P^T             (PE fp32, accumulated over kb;
                                                 row 64 = softmax denominator)
  - O' copied PSUM->SBUF (DVE) and streamed to DRAM out [65, 14336] f32.
Host divides by the denominator row, applies the group normalization
(sum over positions per channel) and the /3, and scatters into the full
(2, 8192, 12, 64) output. Positions not in a dilated group stay zero.
"""

import os
import sys

if "/opt/trn_rl_repo" not in sys.path:
    sys.path.insert(0, "/opt/trn_rl_repo")
if "jax" not in sys.modules:
    os.environ.setdefault("JAX_PLATFORMS", "axon")

import numpy as np

import concourse.bass as bass  # noqa: F401
import concourse.mybir as mybir
import concourse.tile as tile
from concourse import bacc
from concourse.bass_utils import run_bass_kernel_spmd

F32 = mybir.dt.float32
F16 = mybir.dt.float16

B, N, H, D = 2, 8192, 12, 64
NSEG = 7           # segments per core
SEG = 2048         # dilated segment length
NCHUNK = NSEG * 4  # 512-wide q chunks per core
NKB = 16           # 128-row k blocks per segment
NUNIT = NCHUNK * NKB
RW = 3             # k-blocks per exp round (3 PSUM banks per ACT span)
QSC = np.float32(256.0)               # fp16 pre-scale for Q/K splits
ESC = float(0.125 / (256.0 * 256.0))  # exp scale: 1/sqrt(64) + descale

_CACHE = {}
LAST_RESULT = {}


def _build_nc():
    nc = bacc.Bacc("TRN2", target_bir_lowering=False, debug=False,
                   enable_asserts=False, num_devices=8)
    qhh = nc.dram_tensor("qhh", [128, NSEG * SEG], F16, kind="ExternalInput")
    qll = nc.dram_tensor("qll", [128, NSEG * SEG], F16, kind="ExternalInput")
    khl = nc.dram_tensor("khl", [128, NSEG * SEG], F16, kind="ExternalInput")
    v1 = nc.dram_tensor("v1", [128, NSEG * NKB * 65], F32, kind="ExternalInput")
    out = nc.dram_tensor("out", [65, NCHUNK * 512], F32, kind="ExternalOutput")
    qhh_ap, qll_ap, khl_ap, v1_ap, out_ap = (
        qhh.ap(), qll.ap(), khl.ap(), v1.ap(), out.ap())

    with tile.TileContext(nc) as tc:
        with (
            tc.tile_pool(name="inp", bufs=1) as inp,
            tc.tile_pool(name="pt", bufs=4) as ptp,
            tc.tile_pool(name="osb", bufs=3) as osbp,
            tc.tile_pool(name="score", bufs=2, space="PSUM") as scp,
            tc.tile_pool(name="ot", bufs=2, space="PSUM") as otp,
        ):
            qh_sb, ql_sb, k_sb, v1_sb = [], [], [], []
            for s in range(NSEG):
                qh = inp.tile([128, SEG], F16, tag=f"qh{s}", name=f"qh{s}")
                ql = inp.tile([128, SEG], F16, tag=f"ql{s}", name=f"ql{s}")
                kk = inp.tile([128, SEG], F16, tag=f"k{s}", name=f"k{s}")
                vv = inp.tile([128, NKB * 65], F32, tag=f"v{s}", name=f"v{s}")
                sl = slice(s * SEG, (s + 1) * SEG)
                nc.sync.dma_start(qh[:, :], qhh_ap[:, sl])
                nc.sync.dma_start(ql[:, :], qll_ap[:, sl])
                nc.sync.dma_start(kk[:, :], khl_ap[:, sl])
                nc.sync.dma_start(vv[:, :], v1_ap[:, s * NKB * 65:(s + 1) * NKB * 65])
                qh_sb.append(qh)
                ql_sb.append(ql)
                k_sb.append(kk)
                v1_sb.append(vv)

            ot_tiles = {}
            pend = []  # (pt_tile, col, unit) of the previous round

            def flush(items):
                for ptref, i, u in items:
                    cid, kb = divmod(u, NKB)
                    s = cid // 4
                    if kb == 0:
                        ot_tiles[cid] = otp.tile([65, 512], F32, tag="ot",
                                                 name=f"ot{cid}")
                    nc.tensor.matmul(
                        ot_tiles[cid][:, :],
                        v1_sb[s][:, kb * 65:(kb + 1) * 65],
                        ptref[:, i * 512:(i + 1) * 512],
                        start=(kb == 0), stop=(kb == NKB - 1),
                    )
                    if kb == NKB - 1:
                        o_sb = osbp.tile([65, 512], F32, tag="osb",
                                         name=f"osb{cid}")
                        nc.vector.tensor_copy(o_sb[:, :], ot_tiles[cid][:, :])
                        nc.sync.dma_start(
                            out_ap[:, cid * 512:(cid + 1) * 512], o_sb[:, :])

            for r in range((NUNIT + RW - 1) // RW):
                units = range(r * RW, min((r + 1) * RW, NUNIT))
                nu = len(units)
                score = scp.tile([128, 512 * RW], F32, tag="score",
                                 name=f"score{r}")
                for i, u in enumerate(units):
                    cid, kb = divmod(u, NKB)
                    s, c = divmod(cid, 4)
                    osl = slice(i * 512, (i + 1) * 512)
                    csl = slice(c * 512, (c + 1) * 512)
                    lhsT = k_sb[s][:, kb * 128:(kb + 1) * 128]
                    nc.tensor.matmul(score[:, osl], lhsT, qh_sb[s][:, csl],
                                     start=True, stop=False)
                    nc.tensor.matmul(score[:, osl], lhsT, ql_sb[s][:, csl],
                                     start=False, stop=True)
                pt = ptp.tile([128, 512 * RW], F32, tag="pt", name=f"pt{r}")
                nc.scalar.activation(
                    pt[:, :512 * nu], score[:, :512 * nu],
                    mybir.ActivationFunctionType.Exp, scale=ESC)
                flush(pend)
                pend = [(pt, i, u) for i, u in enumerate(units)]
            flush(pend)

    nc.compile()
    return nc


def _prep_core(query, key, value, core):
    b, j = divmod(core, 4)
    segs = []
    for arr in (query, key, value):
        h0 = arr[b, :, j, :].reshape(4, SEG, D)
        h1 = arr[b, :, 4 + j, :].reshape(2, 4096, D)[:, 1::2, :]
        h2 = arr[b, 2::4, 8 + j, :][None]
        segs.append(np.concatenate([h0, h1, h2], axis=0))  # [7, 2048, 64]
    qs, ks, vs = segs
    # [64, NSEG*SEG] with col = s*SEG + p
    qt = (qs * QSC).transpose(2, 0, 1).reshape(D, NSEG * SEG)
    kt = (ks * QSC).transpose(2, 0, 1).reshape(D, NSEG * SEG)
    qh = qt.astype(np.float16)
    ql = (qt - qh).astype(np.float16)
    kh = kt.astype(np.float16)
    kl = (kt - kh).astype(np.float16)
    vv = np.concatenate(
        [vs, np.ones((NSEG, SEG, 1), np.float32)], axis=2)  # [7, 2048, 65]
    v1 = vv.reshape(NSEG, NKB, 128, 65).transpose(2, 0, 1, 3).reshape(128, -1)
    return {
        "qhh": np.ascontiguousarray(np.concatenate([qh, qh], axis=0)),
        "qll": np.ascontiguousarray(np.concatenate([ql, ql], axis=0)),
        "khl": np.ascontiguousarray(np.concatenate([kh, kl], axis=0)),
        "v1": np.ascontiguousarray(v1.astype(np.float32)),
    }


def _unshard(results, dtype):
    full = np.zeros((B, N, H, D), dtype)
    for core in range(8):
        b, j = divmod(core, 4)
        o = results[core]["out"].astype(np.float64)
        T = o[:64] / o[64:65]  # [64, 14336]
        h0 = T[:, :4 * SEG]
        full[b, :, j, :] = (h0 / (3.0 * h0.sum(1, keepdims=True))).T
        h1 = T[:, 4 * SEG:6 * SEG]
        h1 = h1 / (3.0 * h1.sum(1, keepdims=True))
        for g in range(2):
            full[b, g * 4096 + 1:(g + 1) * 4096:2, 4 + j, :] = \
                h1[:, g * SEG:(g + 1) * SEG].T
        h2 = T[:, 6 * SEG:]
        full[b, 2::4, 8 + j, :] = (h2 / (3.0 * h2.sum(1, keepdims=True))).T
    return full


def kernel(query, key, value):
    query = np.asarray(query, np.float32)
    key = np.asarray(key, np.float32)
    value = np.asarray(value, np.float32)
    assert query.shape == (B, N, H, D)

    if "nc" not in _CACHE:
        _CACHE["nc"] = _build_nc()
    nc = _CACHE["nc"]

    in_maps = [_prep_core(query, key, value, c) for c in range(8)]
    res = run_bass_kernel_spmd(nc, in_maps, core_ids=list(range(8)))
    LAST_RESULT["exec_time_ns"] = res.exec_time_ns
    return _unshard(res.results, query.dtype)
